# revision 1
# baseline (speedup 1.0000x reference)
"""Distributed MHA kernel for 8 Trainium2 NeuronCores — v2 (pipelined).

Sharding: core i handles batch b = i//2, head-group g = i%2 (8 of 16 heads).
Restructured for engine overlap relative to the straight-line version:
  - Single strided DMA per tensor load (SP-issue serialization dominated the
    prologue otherwise), ordered so the first QK matmuls start ~5us in and
    the tensor engine is fed as the x/w trickle arrives.
  - Per head-pair m: QK projection, then energy/exp/attV inline per k-block,
    with the NEXT pair's QK matmuls (or stage-C matmuls for the last pair)
    interleaved 2-at-a-time as tensor filler while ScalarE computes exp.
  - exp is one 1024-wide activation per (pair, qs, kblock) covering both
    heads (pse spans two PSUM banks; each matmul writes one bank).
  - 1/(32 den) is broadcast with a tiny matmul: reciprocals are written into
    partitions 0/32 of a zero-padded s2, and ones2^T @ s2 replicates them
    over partitions 0:64/64:128 of the (consumed) po_b psum tile; scale
    planes are staged to SBUF (walrus allows one PSUM operand per DVE op).
  - x/Wq/Wk/Wv are shipped and held in fp16 (halves input DMA; the energy
    is still accumulated in fp32 PSUM and kept in f32r SBUF, so softmax
    precision is preserved); y is written in bf16 (halves output DMA).
    Measured rel err ~3.2e-3 on hardware vs the 2e-2 gate.

Math (per core, heads h in its group, E=1024, H=16, d=64, N=1024):
  QT[hd, n] = sum_e Wq[hd, e] x[n, e] + bq[hd]
  KT[hd, n] = likewise
  V[n, hd]  = sum_e x[n, e] Wv[hd, e]                  (bv folded on host)
  energyT_h[k, q] = sum_d KT_h[d, k] QT_h[d, q]
  expT_h = exp(energyT_h)          (no max-subtract; |energy| < ~50 is safe)
  outT_h[d, q] = sum_k V_h[k, d] expT_h[k, q]; den via ones column
  norm_h[d, q] = outT_h[d, q] * (1/32) * (1/den_h[q])
  y_part[q, e] = sum_{h,d} norm_h[d, q] Wo[e, 64h+d]
Host: out[b] = y_part[2b] + y_part[2b+1] + (bo + Wo @ bv / 32.)
"""

import numpy as np

import concourse.bass as bass
import concourse.tile as tile
from concourse import mybir
from concourse.bass_utils import run_bass_kernel_spmd

E = 1024
N = 1024
B = 4
NC = 8
EH = 512          # head dims per core (8 heads x 64)
D = 64
BF16 = mybir.dt.bfloat16
FP16 = mybir.dt.float16
F32 = mybir.dt.float32
AX = mybir.AluOpType
F32R = mybir.dt.float32r


def split_drain_waits(nc):
    """Walrus in this toolchain rejects instructions carrying more than one
    sem wait; move extra waits onto injected same-engine NOPs placed right
    before the instruction (same engine queue = program order preserved)."""
    def take_nop(engine):
        nop = nc.engines[engine].nop(nofuse=True).ins
        for bname, bw in nc.bb_map.items():
            lst = bw.bb.instructions
            if lst and lst[-1].name == nop.name:
                bw.bb.instructions = lst[:-1]
                break
        return nop

    for name, w in nc.bb_map.items():
        bb = w.bb
        new_insts = []
        changed = False
        for ins in bb.instructions:
            si = ins.sync_info
            if si is not None and si.on_wait and len(si.on_wait) > 1:
                waits = list(si.on_wait)
                for wt in waits[:-1]:
                    nop = take_nop(ins.engine)
                    nop.sync_info = mybir.SyncInfo(on_wait=[wt], on_update=[])
                    new_insts.append(nop)
                si.on_wait = waits[-1:]
                ins.sync_info = si
                changed = True
            new_insts.append(ins)
        if changed:
            bb.instructions = new_insts


def _emit(nc: bass.Bass, tc: tile.TileContext, ctx):
    xT = nc.declare_dram_parameter("xT", [E, N], FP16, isOutput=False)
    wqT = nc.declare_dram_parameter("wqT", [E, EH], FP16, isOutput=False)
    wkT = nc.declare_dram_parameter("wkT", [E, EH], FP16, isOutput=False)
    wvT = nc.declare_dram_parameter("wvT", [E, EH], FP16, isOutput=False)
    woT = nc.declare_dram_parameter("woT", [EH, E], F32R, isOutput=False)
    bqd = nc.declare_dram_parameter("bq", [4, 128, 1], F32, isOutput=False)
    cst = nc.declare_dram_parameter("cst", [64, 640], F32R, isOutput=False)
    bkd = nc.declare_dram_parameter("bk", [4, 128, 1], F32, isOutput=False)
    y = nc.declare_dram_parameter("y", [N, E], BF16, isOutput=True)

    persist = ctx.enter_context(tc.tile_pool(name="persist", bufs=1))
    etp = ctx.enter_context(tc.tile_pool(name="etp", bufs=4))
    sml = ctx.enter_context(tc.tile_pool(name="sml", bufs=2))
    ytr = ctx.enter_context(tc.tile_pool(name="ytr", bufs=3))
    pqk = ctx.enter_context(tc.tile_pool(name="pqk", bufs=1, space="PSUM"))
    pen = ctx.enter_context(tc.tile_pool(name="pen", bufs=2, space="PSUM"))
    pov = ctx.enter_context(tc.tile_pool(name="pov", bufs=1, space="PSUM"))

    # ---- persistent SBUF tiles (e/p as a middle free dim: 1 DMA per load) ----
    xt = persist.tile([128, 8, N], FP16, tag="xt", name="xt")
    wq = persist.tile([128, 8, EH], FP16, tag="wq", name="wq")
    wk = persist.tile([128, 8, EH], FP16, tag="wk", name="wk")
    wv = persist.tile([128, 8, EH], FP16, tag="wv", name="wv")
    wo = persist.tile([128, 4, E], F32R, tag="wo", name="wo")
    qt = [persist.tile([128, N], F32R, tag=f"qt{m}", name=f"qt{m}")
          for m in range(4)]
    kt = [persist.tile([128, N], F32R, tag=f"kt{m}", name=f"kt{m}")
          for m in range(4)]
    # V augmented with a ones column at 64 (gives den for free)
    vt = [persist.tile([128, 8, 65], BF16, tag=f"v{n}", name=f"v{n}")
          for n in range(8)]
    pack = [[persist.tile([128, 512], F32R, tag=f"pk{m}_{qs}", name=f"pk{m}_{qs}")
             for qs in range(2)] for m in range(4)]
    bq_sb = persist.tile([128, 4], F32, tag="bq", name="bq")
    bk_sb = persist.tile([128, 4], F32, tag="bk", name="bk")
    # broadcast-matmul constants: srep = ones2^T @ s2 replicates the two
    # reciprocal rows (partitions 0 and 32 of s2) over partitions 0:64 /
    # 64:128, folding in the 1/32 softmax scale (host-initialized)
    s2 = persist.tile([64, 512], F32R, tag="s2", name="s2")
    ones2 = persist.tile([64, 128], F32R, tag="ones2", name="ones2")

    # ---- DMA issue order (SP engine program order ~ arrival order) ----
    def dma_w_mslice(dst, src, m, eh=None):
        e0, e1 = (0, 8) if eh is None else ((0, 4) if eh == 0 else (4, 8))
        nc.sync.dma_start(
            out=dst[:, e0:e1, m * 128:(m + 1) * 128],
            in_=src[e0 * 128:e1 * 128, m * 128:(m + 1) * 128]
            .rearrange("(e p) c -> p e c", p=128))

    def dma_x_half(h, er=None):
        e0, e1 = er if er is not None else (0, 8)
        nc.sync.dma_start(
            out=xt[:, e0:e1, h * 512:(h + 1) * 512],
            in_=xT[e0 * 128:e1 * 128, h * 512:(h + 1) * 512]
            .rearrange("(e p) n -> p e n", p=128))

    dma_w_mslice(wq, wqT, 0, 0)
    dma_w_mslice(wk, wkT, 0, 0)
    dma_x_half(0, (0, 2))
    dma_x_half(0, (2, 4))
    dma_x_half(0, (4, 6))
    dma_w_mslice(wq, wqT, 0, 1)
    dma_w_mslice(wk, wkT, 0, 1)
    dma_x_half(0, (6, 8))
    nc.sync.dma_start(out=bq_sb, in_=bqd[:, :, :].rearrange("m p one -> p (m one)"))
    nc.sync.dma_start(out=bk_sb, in_=bkd[:, :, :].rearrange("m p one -> p (m one)"))
    nc.sync.dma_start(out=wv, in_=wvT[:, :].rearrange("(e p) c -> p e c", p=128))
    nc.sync.dma_start(out=s2, in_=cst[:, 0:512])
    nc.sync.dma_start(out=ones2, in_=cst[:, 512:640])
    dma_x_half(1, (0, 4))
    dma_x_half(1, (4, 8))
    dma_w_mslice(wq, wqT, 1)
    dma_w_mslice(wk, wkT, 1)
    nc.sync.dma_start(out=wo, in_=woT[:, :].rearrange("(q p) e -> p q e", p=128))
    dma_w_mslice(wq, wqT, 2)
    dma_w_mslice(wk, wkT, 2)
    dma_w_mslice(wq, wqT, 3)
    dma_w_mslice(wk, wkT, 3)

    # ---- compute building blocks ----
    def qk_proj_mms(m, half, which):
        """8 matmuls accumulating W[:,m] @ x[:, half] into a psum tile."""
        w, tag = (wq, "psA") if which == "q" else (wk, "psB")
        ps = pqk.tile([128, 512], F32, tag=tag, name=f"ps_{which}{m}{half}")
        for e in range(8):
            nc.tensor.matmul(
                out=ps, lhsT=(w[:, e, m * 128:(m + 1) * 128]),
                rhs=(xt[:, e, half * 512:(half + 1) * 512]),
                start=(e == 0), stop=(e == 7))
        return ps

    def qk_bias(m, half, which, ps):
        dst, b = (qt, bq_sb) if which == "q" else (kt, bk_sb)
        nc.vector.tensor_scalar_add(
            dst[m][:, half * 512:(half + 1) * 512], ps, b[:, m:m + 1])

    def v_proj(n):
        tv = vt[n]
        ps = pqk.tile([128, 512], F32, tag=("psA", "psB")[n % 2],
                      name=f"psv{n}")
        for e in range(8):
            nc.tensor.matmul(
                out=ps, lhsT=(xt[:, e, n * 128:(n + 1) * 128]), rhs=(wv[:, e, :]),
                start=(e == 0), stop=(e == 7))
        nc.vector.memset(tv[:, :, 64:65], 1.0)
        nc.vector.tensor_copy(
            tv[:, :, 0:64], ps.rearrange("p (h d) -> p h d", h=8))

    # Stage B for pair m, one qs half; filler() emits a couple of independent
    # tensor-engine matmuls between the exp-dependent ones.
    def stage_b(m, qs, po_a, po_b, filler, pre=None, krange=(0, 8)):
        for k in range(*krange):
            pse = pen.tile([128, 1024], F32, tag="pse", name=f"pse{m}{qs}{k}")
            for ab in range(2):
                nc.tensor.matmul(
                    out=pse[:, ab * 512:(ab + 1) * 512],
                    lhsT=(kt[m][ab * 64:(ab + 1) * 64, k * 128:(k + 1) * 128]),
                    rhs=(qt[m][ab * 64:(ab + 1) * 64, qs * 512:(qs + 1) * 512]),
                    start=True, stop=True)
            et = etp.tile([128, 1024], BF16, tag="expT", name=f"et{m}{qs}{k}")
            nc.scalar.activation(
                out=et, in_=pse, func=mybir.ActivationFunctionType.Exp)
            if filler is not None:
                filler()
            if k == 0 and pre is not None:
                pre()
            nc.tensor.matmul(
                out=po_a[0:65], lhsT=vt[k][:, 2 * m, :],
                rhs=et[:, 0:512], start=(k == 0), stop=(k == 7))
            nc.tensor.matmul(
                out=po_b[0:65], lhsT=vt[k][:, 2 * m + 1, :],
                rhs=et[:, 512:1024], start=(k == 0), stop=(k == 7))

    def stage_b_tail_dve(m, qs, po_a, po_b):
        """part 1 (DVE only): reciprocals + stash head-B values."""
        with nc.allow_low_precision(reason="f32r == f32 bits; feeds bcast mm"):
            nc.vector.reciprocal(out=s2[0:1, :], in_=po_a[64:65, :])
            nc.vector.reciprocal(out=s2[32:33, :], in_=po_b[64:65, :])
        tmpb = sml.tile([64, 512], F32, tag="tmpb", name=f"tmpb{m}{qs}")
        nc.vector.tensor_copy(tmpb, po_b[0:64, :])
        return tmpb

    def stage_b_tail_fin(m, qs, po_a, po_b, tmpb):
        """part 2: broadcast matmul + normalize into pack[m][qs]."""
        nc.tensor.matmul(out=po_b[0:128], lhsT=ones2, rhs=s2,
                         start=True, stop=True)
        srepa = sml.tile([64, 512], F32, tag="srepa", name=f"srepa{m}{qs}")
        srepb = sml.tile([64, 512], F32, tag="srepb", name=f"srepb{m}{qs}")
        nc.vector.tensor_copy(srepa, po_b[0:64, :])
        nc.vector.scalar_tensor_tensor(
            out=pack[m][qs][0:64, :], in0=po_a[0:64, :],
            scalar=1.0, in1=srepa, op0=AX.mult, op1=AX.mult)
        nc.vector.tensor_copy(srepb, po_b[64:128, :])
        nc.vector.scalar_tensor_tensor(
            out=pack[m][qs][64:128, :], in0=tmpb,
            scalar=1.0, in1=srepb, op0=AX.mult, op1=AX.mult)

    ys_open = {}

    def stage_c_out(qs, qq, es, pstile):
        """copy chunk into the row-block staging tile; DMA once per qt_i."""
        qt_i = qs * 4 + qq
        if es == 0:
            ys_open[qt_i] = ytr.tile([128, 1024], BF16, tag="ysb",
                                     name=f"ys{qs}{qq}")
        ys = ys_open[qt_i]
        nc.vector.tensor_copy(ys[:, es * 512:(es + 1) * 512], pstile)
        if qt_i == 7:
            nc.sync.dma_start(
                out=y[qt_i * 128:(qt_i + 1) * 128, es * 512:(es + 1) * 512],
                in_=ys[:, es * 512:(es + 1) * 512])
        elif es == 1:
            nc.sync.dma_start(
                out=y[qt_i * 128:(qt_i + 1) * 128, :], in_=ys)

    def stage_c(qs, qq, es, pstile=None):
        if pstile is None:
            pstile = pqk.tile([128, 512], F32, tag=("psA", "psB")[es],
                              name=f"psy{qs}{qq}{es}")
        for p in range(4):
            nc.tensor.matmul(
                out=pstile, lhsT=(pack[p][qs][:, qq * 128:(qq + 1) * 128]),
                rhs=(wo[:, p, es * 512:(es + 1) * 512]),
                start=(p == 0), stop=(p == 3))
        stage_c_out(qs, qq, es, pstile)

    # ---- schedule ----
    # QK pair 0, half 0: interleave Q and K e-pairs so the tensor engine
    # consumes the x/w DMA trickle as it arrives
    psq = pqk.tile([128, 512], F32, tag="psA", name="ps_q00")
    psk = pqk.tile([128, 512], F32, tag="psB", name="ps_k00")
    for e0 in (0, 2, 4, 6):
        for e in (e0, e0 + 1):
            nc.tensor.matmul(
                out=psq, lhsT=(wq[:, e, 0:128]), rhs=(xt[:, e, 0:512]),
                start=(e == 0), stop=(e == 7))
        for e in (e0, e0 + 1):
            nc.tensor.matmul(
                out=psk, lhsT=(wk[:, e, 0:128]), rhs=(xt[:, e, 0:512]),
                start=(e == 0), stop=(e == 7))
    qk_bias(0, 0, "q", psq)
    qk_bias(0, 0, "k", psk)
    for n in range(4):
        v_proj(n)
    # first half of stage B(0, qs0) only needs half-0 Q/K and V blocks 0-3:
    # fills the tensor hole while x half1 is still in flight
    po_a0 = pov.tile([128, 512], F32, tag="poa", name="poa00")
    po_b0 = pov.tile([128, 512], F32, tag="pob", name="pob00")
    stage_b(0, 0, po_a0, po_b0, None, krange=(0, 4))
    ps = qk_proj_mms(0, 1, "q"); qk_bias(0, 1, "q", ps)
    ps = qk_proj_mms(0, 1, "k"); qk_bias(0, 1, "k", ps)
    for n in range(4, 8):
        v_proj(n)

    # pairs: stage B(m) with QK(m+1) / stage C as tensor filler.
    # Filler emits ~2 matmuls per call so the tensor queue always has a
    # little independent work while exp runs on the scalar engine.
    class Filler:
        def __init__(self):
            self.steps = []  # list of closures, each ~2 matmuls

        def add_qk(self, m):
            for which in ("q", "k"):
                for half in range(2):
                    st = {}

                    def open_(m=m, half=half, which=which, st=st):
                        w = wq if which == "q" else wk
                        tag = "psA" if which == "q" else "psB"
                        st["ps"] = pqk.tile([128, 512], F32, tag=tag,
                                            name=f"ps_{which}{m}{half}")

                    def emit(e0, m=m, half=half, which=which, st=st, open_=open_):
                        if e0 == 0:
                            open_()
                        w = wq if which == "q" else wk
                        for e in (e0, e0 + 1):
                            nc.tensor.matmul(
                                out=st["ps"],
                                lhsT=(w[:, e, m * 128:(m + 1) * 128]),
                                rhs=(xt[:, e, half * 512:(half + 1) * 512]),
                                start=(e == 0), stop=(e == 7))
                        if e0 == 6:
                            qk_bias(m, half, which, st["ps"])

                    for e0 in (0, 2, 4, 6):
                        self.steps.append(lambda e0=e0, emit=emit: emit(e0))

        def add_c(self, qs):
            for qq in range(4):
                for es in range(2):
                    st = {}

                    def emit(p0, qs=qs, qq=qq, es=es, st=st):
                        if p0 == 0:
                            st["ps"] = pqk.tile(
                                [128, 512], F32, tag=("psA", "psB")[es],
                                name=f"psy{qs}{qq}{es}")
                        for p in (p0, p0 + 1):
                            nc.tensor.matmul(
                                out=st["ps"],
                                lhsT=(pack[p][qs][:, qq * 128:(qq + 1) * 128]),
                                rhs=(wo[:, p, es * 512:(es + 1) * 512]),
                                start=(p == 0), stop=(p == 3))
                        if p0 == 2:
                            stage_c_out(qs, qq, es, st["ps"])

                    for p0 in (0, 2):
                        self.steps.append(lambda p0=p0, emit=emit: emit(p0))

        def __call__(self):
            if self.steps:
                self.steps.pop(0)()

    fill = Filler()
    pending = None
    for m in range(4):
        if m < 3:
            fill.add_qk(m + 1)
        for qs in range(2):
            if m == 3 and qs == 1:
                fill.add_c(0)
            if m == 0 and qs == 0:
                po_a, po_b = po_a0, po_b0
                stage_b(m, qs, po_a, po_b, fill, krange=(4, 8))
            else:
                po_a = pov.tile([128, 512], F32, tag="poa", name=f"poa{m}{qs}")
                po_b = pov.tile([128, 512], F32, tag="pob", name=f"pob{m}{qs}")
                stage_b(m, qs, po_a, po_b, fill, pre=pending)
            tmpb = stage_b_tail_dve(m, qs, po_a, po_b)
            pending = (lambda m=m, qs=qs, po_a=po_a, po_b=po_b, tmpb=tmpb:
                       stage_b_tail_fin(m, qs, po_a, po_b, tmpb))
    pending()
    while fill.steps:
        fill()
    # final C half: rotate over 4 independent psum banks (psA, psB, and the
    # now-free attV tiles poa/pob) so y DMAs overlap the remaining matmuls
    for qq in range(4):
        for es in range(2):
            idx = qq * 2 + es
            if idx % 4 == 2:
                pt = pov.tile([128, 512], F32, tag="poa", name=f"psyc{qq}a")
                stage_c(1, qq, es, pt)
            elif idx % 4 == 3:
                pt = pov.tile([128, 512], F32, tag="pob", name=f"psyc{qq}b")
                stage_c(1, qq, es, pt)
            else:
                stage_c(1, qq, es)


def build(apply_walrus_fix=True):
    from contextlib import ExitStack
    nc = bass.Bass()
    with tile.TileContext(nc) as tc:
        with ExitStack() as ctx:
            _emit(nc, tc, ctx)
    if apply_walrus_fix:
        split_drain_waits(nc)
    return nc


def make_in_maps(x, Wq, bq, Wk, bk, Wv, bv, Wo, bo):
    cst_arr = np.zeros((64, 640), np.float32)
    cst_arr[0, 512:576] = 1.0 / 32.0
    cst_arr[32, 576:640] = 1.0 / 32.0
    in_maps = []
    for i in range(NC):
        b, g = i // 2, i % 2
        sl = slice(g * EH, (g + 1) * EH)
        in_maps.append({
            "xT": np.ascontiguousarray(x[b].T).astype(np.float16),
            "wqT": np.ascontiguousarray(Wq[sl, :].T).astype(np.float16),
            "wkT": np.ascontiguousarray(Wk[sl, :].T).astype(np.float16),
            "wvT": np.ascontiguousarray(Wv[sl, :].T).astype(np.float16),
            "woT": np.ascontiguousarray(Wo[:, sl].T),
            "bq": bq[sl].reshape(4, 128, 1).astype(np.float32),
            "bk": bk[sl].reshape(4, 128, 1).astype(np.float32),
            "cst": cst_arr,
        })
    return in_maps


def gather(results, Wv_b, Wo, bv, bo):
    host_bias = (bo + Wo @ bv / 32.0).astype(np.float32)
    out = np.empty((B, N, E), np.float32)
    for b in range(B):
        out[b] = (results[2 * b]["y"].astype(np.float32)
                  + results[2 * b + 1]["y"].astype(np.float32) + host_bias)
    return out


def kernel(x, Wq, bq, Wk, bk, Wv, bv, Wo, bo):
    x, Wq, bq, Wk, bk, Wv, bv, Wo, bo = [
        np.asarray(a, np.float32) for a in (x, Wq, bq, Wk, bk, Wv, bv, Wo, bo)]
    nc = build()
    in_maps = make_in_maps(x, Wq, bq, Wk, bk, Wv, bv, Wo, bo)
    res = run_bass_kernel_spmd(nc, in_maps, list(range(NC)))
    return gather(res.results, Wv, Wo, bv, bo)


if __name__ == "__main__":
    import reference
    inputs = {k: np.asarray(v) for k, v in reference.setup_inputs().items()}
    out = kernel(**inputs)
    exp = np.asarray(reference.reference(**inputs))
    rel = np.abs(out - exp).max() / np.abs(exp).max()
    print("Relative error:", rel)



# revision 18
# speedup vs baseline: 3.2769x; 3.2769x over previous
"""Distributed MHA kernel for 8 Trainium2 NeuronCores — v3 (phase-pipelined).

Sharding: core i handles batch b = i//2, head-group g = i%2 (8 of 16 heads).

v3 restructures v2 around measured per-structure economics of this hardware
(pure 512-wide MM streams run at ~264 ns/MM incl. LDWEIGHTS and semaphores;
tight MM->exp->MM chains and heterogeneous interleaving ran ~3x over the
cost model in v2):
  - Software pipelining at the head-pair level: energy/exp for pair m+1 is
    emitted interleaved with attV for pair m, so no tensor-engine matmul
    ever waits on the exp activation produced in the same phase. et tiles
    for two pairs are kept in SBUF (64 KB) to decouple the stages.
  - LDWEIGHTS sharing: QKV projections emit both n-halves per weight slice;
    attV loads each vt slice once for both qs halves; stage C loads each
    pack slice once for both output halves.
  - qt/kt held in FP16 (halves their SBUF, feeds 16-bit-moving energy MMs;
    energy still accumulates in fp32 PSUM, so softmax precision holds).
  - PSUM: pen pool = double-buffered [128,1024] energy tiles (4 banks),
    pov pool = 4 single-bank tiles that serve as QKV accumulators in the
    prologue, the 4 attV accumulators in the steady state, and the stage-C
    accumulators in the tail.
  - x/Wq/Wk/Wv shipped fp16, y written bf16. Measured v2 rel err 3.2e-3;
    v3 adds only fp16 qt/kt rounding (emulated ~4e-3) vs the 2e-2 gate.

Math (per core, heads h in its group, E=1024, H=16, d=64, N=1024):
  QT[hd, n] = sum_e Wq[hd, e] x[n, e] + bq[hd]    (KT likewise)
  V[n, hd]  = sum_e x[n, e] Wv[hd, e]             (bv folded on host)
  energyT_h[k, q] = sum_d KT_h[d, k] QT_h[d, q]
  expT_h = exp(energyT_h)          (no max-subtract; |energy| < ~70 is safe)
  outT_h[d, q] = sum_k V_h[k, d] expT_h[k, q]; den via ones column
  norm_h[d, q] = outT_h[d, q] * (1/32) * (1/den_h[q])
  y_part[q, e] = sum_{h,d} norm_h[d, q] Wo[e, 64h+d]
Host: out[b] = y_part[2b] + y_part[2b+1] + (bo + Wo @ bv / 32.)
"""

import numpy as np

import concourse.bass as bass
import concourse.tile as tile
from concourse import mybir
from concourse.bass_utils import run_bass_kernel_spmd

E = 1024
N = 1024
B = 4
NC = 8
EH = 512          # head dims per core (8 heads x 64)
D = 64
BF16 = mybir.dt.bfloat16
FP16 = mybir.dt.float16
F32 = mybir.dt.float32
AX = mybir.AluOpType
F32R = mybir.dt.float32r
ABLATE = {}  # experiment switches; empty in production


def split_drain_waits(nc):
    """Walrus in this toolchain rejects instructions carrying more than one
    sem wait; move extra waits onto injected same-engine NOPs placed right
    before the instruction (same engine queue = program order preserved)."""
    def take_nop(engine):
        nop = nc.engines[engine].nop(nofuse=True).ins
        for bname, bw in nc.bb_map.items():
            lst = bw.bb.instructions
            if lst and lst[-1].name == nop.name:
                bw.bb.instructions = lst[:-1]
                break
        return nop

    for name, w in nc.bb_map.items():
        bb = w.bb
        new_insts = []
        changed = False
        for ins in bb.instructions:
            si = ins.sync_info
            if si is not None and si.on_wait and len(si.on_wait) > 1:
                waits = list(si.on_wait)
                for wt in waits[:-1]:
                    nop = take_nop(ins.engine)
                    nop.sync_info = mybir.SyncInfo(on_wait=[wt], on_update=[])
                    new_insts.append(nop)
                si.on_wait = waits[-1:]
                ins.sync_info = si
                changed = True
            new_insts.append(ins)
        if changed:
            bb.instructions = new_insts


def _emit(nc: bass.Bass, tc: tile.TileContext, ctx, repeats=1):
    xT = nc.declare_dram_parameter("xT", [E, N], FP16, isOutput=False)
    wqT = nc.declare_dram_parameter("wqT", [E, EH], FP16, isOutput=False)
    wkT = nc.declare_dram_parameter("wkT", [E, EH], FP16, isOutput=False)
    wvT = nc.declare_dram_parameter("wvT", [E, EH], FP16, isOutput=False)
    woT = nc.declare_dram_parameter("woT", [EH, E], BF16, isOutput=False)
    bqd = nc.declare_dram_parameter("bq", [4, 128, 1], F32, isOutput=False)
    cst = nc.declare_dram_parameter("cst", [64, 1152], F32R, isOutput=False)
    bkd = nc.declare_dram_parameter("bk", [4, 128, 1], F32, isOutput=False)
    y = nc.declare_dram_parameter("y", [N, E], BF16, isOutput=True)

    persist = ctx.enter_context(tc.tile_pool(name="persist", bufs=1))
    sml = ctx.enter_context(tc.tile_pool(name="sml", bufs=2))
    ytr = ctx.enter_context(tc.tile_pool(name="ytr", bufs=3))
    pen = ctx.enter_context(tc.tile_pool(name="pen", bufs=2, space="PSUM"))
    pov = ctx.enter_context(tc.tile_pool(name="pov", bufs=1, space="PSUM"))

    # ---- persistent SBUF tiles ----
    xt = persist.tile([128, 8, N], FP16, tag="xt", name="xt")
    wq = persist.tile([128, 8, EH], FP16, tag="wq", name="wq")
    wk = persist.tile([128, 8, EH], FP16, tag="wk", name="wk")
    wv = persist.tile([128, 8, EH], FP16, tag="wv", name="wv")
    wo = persist.tile([128, 4, E], BF16, tag="wo", name="wo")
    qt = [persist.tile([128, N], FP16, tag=f"qt{m}", name=f"qt{m}")
          for m in range(4)]
    kt = [persist.tile([128, N], FP16, tag=f"kt{m}", name=f"kt{m}")
          for m in range(4)]
    # V augmented with a ones column at 64 (gives den for free)
    vt = [persist.tile([128, 8, 65], BF16, tag=f"v{n}", name=f"v{n}")
          for n in range(8)]
    # exp tiles for two pipeline stages (pair m and m+1): [qs][k] each
    et2 = [[[persist.tile([128, 1024], BF16, tag=f"et{p}_{qs}{k}",
                          name=f"et{p}_{qs}{k}")
             for k in range(8)] for qs in range(2)] for p in range(2)]
    pack = [[persist.tile([128, 512], BF16, tag=f"pk{m}_{qs}", name=f"pk{m}_{qs}")
             for qs in range(2)] for m in range(4)]
    bq_sb = persist.tile([128, 4], F32, tag="bq", name="bq")
    bk_sb = persist.tile([128, 4], F32, tag="bk", name="bk")
    # broadcast-matmul constants: srep = ones2^T @ s2[:, qs*512:] replicates
    # the reciprocal rows (partitions 0/32 of s2) over partitions 0:64 /
    # 64:128, folding in the 1/32 softmax scale. ones2 rows != {0,32} are
    # zero; s2 is still zero-loaded from cst so no NaNs enter the product.
    s2 = persist.tile([64, 1024], F32R, tag="s2", name="s2")
    ones2 = persist.tile([64, 128], F32R, tag="ones2", name="ones2")

    # ---- DMA helpers (one strided DMA per chunk; SP order ~ arrival) ----
    def dma_w_mslice(dst, src, m, eh=None):
        e0, e1 = (0, 8) if eh is None else ((0, 4) if eh == 0 else (4, 8))
        nc.sync.dma_start(
            out=dst[:, e0:e1, m * 128:(m + 1) * 128],
            in_=src[e0 * 128:e1 * 128, m * 128:(m + 1) * 128]
            .rearrange("(e p) c -> p e c", p=128))

    def dma_x(er):
        e0, e1 = er
        nc.sync.dma_start(
            out=xt[:, e0:e1, :],
            in_=xT[e0 * 128:e1 * 128, :]
            .rearrange("(e p) n -> p e n", p=128))

    def issue_dmas():
        dma_w_mslice(wq, wqT, 0, 0)
        dma_w_mslice(wk, wkT, 0, 0)
        dma_x((0, 1))
        dma_x((1, 2))
        dma_x((2, 3))
        dma_w_mslice(wq, wqT, 0, 1)
        dma_w_mslice(wk, wkT, 0, 1)
        dma_x((3, 4))
        nc.sync.dma_start(out=bq_sb, in_=bqd[:, :, :].rearrange("m p one -> p (m one)"))
        nc.sync.dma_start(out=bk_sb, in_=bkd[:, :, :].rearrange("m p one -> p (m one)"))
        dma_x((4, 6))
        nc.sync.dma_start(out=wv, in_=wvT[:, :].rearrange("(e p) c -> p e c", p=128))
        nc.sync.dma_start(out=s2, in_=cst[:, 0:1024])
        nc.sync.dma_start(out=ones2, in_=cst[:, 1024:1152])
        dma_x((6, 8))
        dma_w_mslice(wq, wqT, 1)
        dma_w_mslice(wk, wkT, 1)
        nc.sync.dma_start(out=wo, in_=woT[:, :].rearrange("(q p) e -> p q e", p=128))
        dma_w_mslice(wq, wqT, 2)
        dma_w_mslice(wk, wkT, 2)
        dma_w_mslice(wq, wqT, 3)
        dma_w_mslice(wk, wkT, 3)

    # ---- compute building blocks ----
    def qk_bias(m, half, which, ps):
        dst, b = (qt, bq_sb) if which == "q" else (kt, bk_sb)
        with nc.allow_low_precision(reason="qt/kt held fp16; |q|,|k| ~ O(5)"):
            nc.vector.tensor_scalar_add(
                dst[m][:, half * 512:(half + 1) * 512], ps, b[:, m:m + 1])

    def qk_proj_steps(m, which):
        """generator of per-e steps: each weight slice loaded once, used for
        both n-halves (psum tags pA/pB)."""
        w = wq if which == "q" else wk
        st = {}

        def emit(e):
            if e == 0:
                st["A"] = pov.tile([128, 512], F32, tag="pA",
                                   name=f"ps_{which}{m}0")
                st["B"] = pov.tile([128, 512], F32, tag="pB",
                                   name=f"ps_{which}{m}1")
            for half, ps in ((0, st["A"]), (1, st["B"])):
                nc.tensor.matmul(
                    out=ps, lhsT=(w[:, e, m * 128:(m + 1) * 128]),
                    rhs=(xt[:, e, half * 512:(half + 1) * 512]),
                    start=(e == 0), stop=(e == 7))
            if e == 7:
                qk_bias(m, 0, which, st["A"])
                qk_bias(m, 1, which, st["B"])

        return [lambda e=e: emit(e) for e in range(8)]

    def v_proj_steps(n):
        tv = vt[n]
        st = {}

        def emit(e):
            if e == 0:
                st["ps"] = pov.tile([128, 512], F32, tag=("pC", "pD")[n % 2],
                                    name=f"psv{n}")
            nc.tensor.matmul(
                out=st["ps"], lhsT=(xt[:, e, n * 128:(n + 1) * 128]),
                rhs=(wv[:, e, :]),
                start=(e == 0), stop=(e == 7))
            if e == 7:
                nc.vector.memset(tv[:, :, 64:65], 1.0)
                nc.vector.tensor_copy(
                    tv[:, :, 0:64], st["ps"].rearrange("p (h d) -> p h d", h=8))

        return [lambda e=e: emit(e) for e in range(8)]

    def en_steps(m):
        """energy+exp for pair m into et2[m % 2]: per (qs, k) one step =
        2 matmuls (heads share nothing; kt slices differ) + 1 exp."""
        def emit(qs, k):
            pse = pen.tile([128, 1024], F32, tag="pse", name=f"pse{m}{qs}{k}")
            for ab in range(2):
                nc.tensor.matmul(
                    out=pse[:, ab * 512:(ab + 1) * 512],
                    lhsT=(kt[m][ab * 64:(ab + 1) * 64, k * 128:(k + 1) * 128]),
                    rhs=(qt[m][ab * 64:(ab + 1) * 64,
                               qs * 512:(qs + 1) * 512]),
                    start=True, stop=True)
            nc.scalar.activation(
                out=et2[m % 2][qs][k], in_=pse,
                func=mybir.ActivationFunctionType.Exp)

        return [lambda qs=qs, k=k: emit(qs, k)
                for k in range(8) for qs in range(2)]

    def av_steps(m, po):
        """attV for pair m from et2[m % 2]: per k one step = 4 matmuls;
        each vt slice loaded once for both qs halves."""
        ets = et2[m % 2]

        def emit(k):
            for ab in range(2):
                lhs = vt[k][:, 2 * m + ab, :]
                for qs in range(2):
                    nc.tensor.matmul(
                        out=po[2 * ab + qs][0:65], lhsT=lhs,
                        rhs=ets[qs][k][:, ab * 512:(ab + 1) * 512],
                        start=(k == 0), stop=(k == 7))

        return [lambda k=k: emit(k) for k in range(8)]

    def tail_dve(m, po):
        """reciprocals of the den rows + stash head-B planes to SBUF."""
        with nc.allow_low_precision(reason="f32r == f32 bits; feeds bcast mm"):
            nc.vector.reciprocal(out=s2[0:1, 0:512], in_=po[0][64:65, :])
            nc.vector.reciprocal(out=s2[0:1, 512:1024], in_=po[1][64:65, :])
            nc.vector.reciprocal(out=s2[32:33, 0:512], in_=po[2][64:65, :])
            nc.vector.reciprocal(out=s2[32:33, 512:1024], in_=po[3][64:65, :])
        tb = [sml.tile([64, 512], F32, tag=f"tmp{qs}", name=f"tmpb{m}{qs}")
              for qs in range(2)]
        nc.vector.tensor_copy(tb[0], po[2][0:64, :])
        nc.vector.tensor_copy(tb[1], po[3][0:64, :])
        return tb

    def tail_fin(m, po, tb):
        """broadcast matmuls + normalize into pack[m][qs]. The bcast output
        borrows a pen-pool slot (pse tag) — po banks are all accumulators."""
        for qs in range(2):
            bc = pen.tile([128, 512], F32, tag="pse", name=f"bc{m}{qs}")
            nc.tensor.matmul(out=bc, lhsT=ones2,
                             rhs=s2[:, qs * 512:(qs + 1) * 512],
                             start=True, stop=True)
            srepa = sml.tile([64, 512], F32, tag="srepa", name=f"srepa{m}{qs}")
            srepb = sml.tile([64, 512], F32, tag="srepb", name=f"srepb{m}{qs}")
            nc.vector.tensor_copy(srepa, bc[0:64, :])
            nc.vector.tensor_copy(srepb, bc[64:128, :])
            with nc.allow_low_precision(reason="pack held bf16; |v|~0.03"):
                nc.vector.scalar_tensor_tensor(
                    out=pack[m][qs][0:64, :], in0=po[qs][0:64, :],
                    scalar=1.0, in1=srepa, op0=AX.mult, op1=AX.mult)
                nc.vector.scalar_tensor_tensor(
                    out=pack[m][qs][64:128, :], in0=tb[qs],
                    scalar=1.0, in1=srepb, op0=AX.mult, op1=AX.mult)

    ys_open = {}

    def stage_c_out(qs, qq, es, pstile):
        qt_i = qs * 4 + qq
        if es == 0:
            ys_open[qt_i] = ytr.tile([128, 1024], BF16, tag="ysb",
                                     name=f"ys{qs}{qq}")
        ys = ys_open[qt_i]
        nc.vector.tensor_copy(ys[:, es * 512:(es + 1) * 512], pstile)
        if qt_i == 7:
            nc.sync.dma_start(
                out=y[qt_i * 128:(qt_i + 1) * 128, es * 512:(es + 1) * 512],
                in_=ys[:, es * 512:(es + 1) * 512])
        elif es == 1:
            nc.sync.dma_start(
                out=y[qt_i * 128:(qt_i + 1) * 128, :], in_=ys)

    def stage_c(qs, qq, tags):
        """both es halves per pack slice: each lhsT loaded once."""
        psA = pov.tile([128, 512], F32, tag=tags[0], name=f"psy{qs}{qq}0")
        psB = pov.tile([128, 512], F32, tag=tags[1], name=f"psy{qs}{qq}1")
        for p in range(4):
            for es, ps in ((0, psA), (1, psB)):
                nc.tensor.matmul(
                    out=ps, lhsT=(pack[p][qs][:, qq * 128:(qq + 1) * 128]),
                    rhs=(wo[:, p, es * 512:(es + 1) * 512]),
                    start=(p == 0), stop=(p == 3))
        stage_c_out(qs, qq, 0, psA)
        stage_c_out(qs, qq, 1, psB)

    def interleave(primary, filler, ratio):
        """emit primary steps, injecting `ratio` filler steps after each."""
        for step in primary:
            step()
            for _ in range(ratio):
                if filler:
                    filler.pop(0)()
        while filler:
            filler.pop(0)()

    # ---- schedule ----
    def schedule():
        # P1: QK(0) consumes the x/w trickle as it arrives
        for step in qk_proj_steps(0, "q") + qk_proj_steps(0, "k"):
            step()
        # en(0) (ACT-heavy) interleaved with remaining QKV (PE-heavy)
        rest = []
        for n in range(8):
            rest += v_proj_steps(n)
        for m in (1, 2, 3):
            rest += qk_proj_steps(m, "q") + qk_proj_steps(m, "k")
        interleave(en_steps(0), rest, 7)

        # steady state: attV(m) + tail(m) interleaved with energy/exp(m+1)
        pending = None
        for m in range(4):
            po = [pov.tile([128, 512], F32, tag=t, name=f"po{m}{t}")
                  for t in ("pA", "pB", "pC", "pD")]
            filler = en_steps(m + 1) if m < 3 else []
            avs = av_steps(m, po)
            if pending is not None:
                pending()
            for i, step in enumerate(avs):
                step()
                for _ in range(2):
                    if filler:
                        filler.pop(0)()
            while filler:
                filler.pop(0)()
            tb = tail_dve(m, po)
            pending = (lambda m=m, po=po, tb=tb: tail_fin(m, po, tb))
        pending()

        # stage C: pack @ Wo, alternating psum tag pairs so y DMAs overlap
        for qs in range(2):
            for qq in range(4):
                tags = (("pA", "pB"), ("pC", "pD"))[qq % 2]
                stage_c(qs, qq, tags)

    for _rep in range(repeats):
        issue_dmas()
        schedule()


def build(apply_walrus_fix=True, repeats=1):
    from contextlib import ExitStack
    nc = bass.Bass()
    with tile.TileContext(nc) as tc:
        with ExitStack() as ctx:
            _emit(nc, tc, ctx, repeats=repeats)
    if apply_walrus_fix:
        split_drain_waits(nc)
    return nc


def _bf16(a):
    import jax.numpy as jnp
    return np.asarray(jnp.asarray(a, dtype="bfloat16"))


def make_in_maps(x, Wq, bq, Wk, bk, Wv, bv, Wo, bo):
    cst_arr = np.zeros((64, 1152), np.float32)
    cst_arr[0, 1024:1088] = 1.0 / 32.0
    cst_arr[32, 1088:1152] = 1.0 / 32.0
    in_maps = []
    for i in range(NC):
        b, g = i // 2, i % 2
        sl = slice(g * EH, (g + 1) * EH)
        in_maps.append({
            "xT": np.ascontiguousarray(x[b].T).astype(np.float16),
            "wqT": np.ascontiguousarray(Wq[sl, :].T).astype(np.float16),
            "wkT": np.ascontiguousarray(Wk[sl, :].T).astype(np.float16),
            "wvT": np.ascontiguousarray(Wv[sl, :].T).astype(np.float16),
            "woT": np.ascontiguousarray(Wo[:, sl].T).astype(
                np.dtype("bfloat16") if hasattr(np, "bfloat16") else np.float32)
            if False else _bf16(np.ascontiguousarray(Wo[:, sl].T)),
            "bq": bq[sl].reshape(4, 128, 1).astype(np.float32),
            "bk": bk[sl].reshape(4, 128, 1).astype(np.float32),
            "cst": cst_arr,
        })
    return in_maps


def gather(results, Wv_b, Wo, bv, bo):
    host_bias = (bo + Wo @ bv / 32.0).astype(np.float32)
    out = np.empty((B, N, E), np.float32)
    for b in range(B):
        out[b] = (results[2 * b]["y"].astype(np.float32)
                  + results[2 * b + 1]["y"].astype(np.float32) + host_bias)
    return out


def kernel(x, Wq, bq, Wk, bk, Wv, bv, Wo, bo):
    x, Wq, bq, Wk, bk, Wv, bv, Wo, bo = [
        np.asarray(a, np.float32) for a in (x, Wq, bq, Wk, bk, Wv, bv, Wo, bo)]
    nc = build()
    in_maps = make_in_maps(x, Wq, bq, Wk, bk, Wv, bv, Wo, bo)
    res = run_bass_kernel_spmd(nc, in_maps, list(range(NC)))
    return gather(res.results, Wv, Wo, bv, bo)


if __name__ == "__main__":
    import reference
    inputs = {k: np.asarray(v) for k, v in reference.setup_inputs().items()}
    out = kernel(**inputs)
    exp = np.asarray(reference.reference(**inputs))
    rel = np.abs(out - exp).max() / np.abs(exp).max()
    print("Relative error:", rel)
